# revision 20
# baseline (speedup 1.0000x reference)
"""Trainium2 Bass kernel for pre-LN multi-head self-attention (v4).

Problem shapes (hardcoded): q (4, 2048, 1024) f32, attn_mask (2048, 2048) bool,
Wq/Wk/Wv (1024, 1024) f32, Wo (1024, 1024) f32, gamma/beta (1024,) f32.
N_HEAD=16, D_HEAD=64, pre-layernorm, softmax over the key axis.

Sharding: 8 cores = 4 batches x 2 head-groups (8 heads each). Host sums the
two head-group partials plus qn per batch.

v4 design (vs v2 baseline at 475us):
  - Everything fp8: scores, AV (DoubleRow over jc pairs), O-proj (DoubleRow).
  - hq/hk quantized with alpha = sqrt(8*log2(e)*SCALE) so the score psum is
    exactly 8*log2(e)*s -- the Schraudolph constant for fp8e4m3.  The DVE
    softmax unit is then ONE scalar_tensor_tensor: uint8(psum + 23.549)*mask,
    whose saturating-uint8 result bytes ARE fp8 probabilities exp(s)/16.
  - ACT softmax units run a real Exp to fp8; their mask is pre-applied into
    the score psum by a DoubleRow matmul with lhsT = -128*I and rhs = the
    same fp8 0/1 mask tile (213ns on the PE, no DVE involvement).
  - AV accumulates jc-PAIRS per DoubleRow matmul (pa packed [128,2,512]
    uint8-bitcast-fp8); hv carries a ones column => psum row 64 = denom.
  - Normalization after AV: recip via DRAM-broadcast trick (as v2), vecT is
    fp8 scaled x16; Wo x32; final psum->bf16 copy scales by 1/512.
  - Outputs bf16 (qn + per-core partial); host does the residual sum in f32.
"""

import numpy as np
import ml_dtypes
from contextlib import ExitStack

import concourse.bass as bass
import concourse.tile as tile
from concourse import bacc, mybir
from concourse.bass_utils import run_bass_kernel_spmd

F32 = mybir.dt.float32
BF16 = mybir.dt.bfloat16
FP8 = mybir.dt.float8e4
U8 = mybir.dt.uint8
Alu = mybir.AluOpType
Act = mybir.ActivationFunctionType
DR = mybir.MatmulPerfMode.DoubleRow

BSZ, SEQ, DM = 4, 2048, 1024
NH, DH = 16, 64
HPC = 8              # heads per core
HD = HPC * DH        # 512 = per-core slice of the head dim
NCORES = 8
SCALE = 1.0 / (DH ** 0.5)
LN_EPS = 1e-5

NT = SEQ // 16 // 8  # placeholder (unused)
NJC = SEQ // 128     # 16 j-tiles
NDC = DM // 128      # 8 d_model chunks
K_SCH = 8.0 / np.log(2.0)          # 11.541560: psum = K_SCH * s
ALPHA = float(np.sqrt(K_SCH * SCALE))   # 1.2011224 hq/hk quant scale
B8 = 23.549                         # Schraudolph offset (DVE stt scalar)
BIAS_ACT = float(-np.log(16.0) - 128.0 * np.log(2.0) / 8.0)  # -ln16 - 128/K
SCALE_ACT = float(1.0 / K_SCH)
HVP = 72                            # padded hv row stride (16B-aligned e-step)

# per-16-jc engine pattern: True = DVE fused unit, False = ACT unit
DVE_JC = (0, 1, 2, 4, 5, 6, 8, 9, 10, 12, 14)


def _mha_tile(ctx, tc, dq, dmask, dwq, dwk, dwv, dwo, dident, dnegid,
              dgamma, dbeta, dqn, dpart, drecip):
    nc = tc.nc

    persist = ctx.enter_context(tc.tile_pool(name="persist", bufs=1))
    identb = persist.tile([128, 128], BF16)
    nc.sync.dma_start(out=identb, in_=dident)
    negid = persist.tile([128, 2, 128], FP8)
    nc.sync.dma_start(out=negid, in_=dnegid)
    eps_sb = persist.tile([128, 1], F32)
    nc.vector.memset(eps_sb, LN_EPS)
    bias_sb = persist.tile([128, 1], F32)
    nc.vector.memset(bias_sb, BIAS_ACT)

    hqT = persist.tile([128, 4, SEQ], FP8)     # [d-in-chunk, hc, i]
    hkT = persist.tile([128, 4, SEQ], FP8)
    qnTall = persist.tile([128, 4, 4, 2, 512], FP8)  # [dm-chunk, tb, c, e, i]
    wq_sb = persist.tile([128, 4, 2, HD], FP8)
    wk_sb = persist.tile([128, 4, 2, HD], FP8)
    hv2 = persist.tile([128, NJC // 2, 2, HPC, HVP], FP8)  # [j, jcp, e, h, d]
    mask01 = persist.tile([128, NJC, SEQ], FP8)  # [j-part, jc, i] 0/1
    wo2 = persist.tile([128, 2, 2, DM], FP8)   # [p, half, e, m]

    # big persistent loads spread across the otherwise-idle DMA queues
    for c in range(8):
        nc.gpsimd.dma_start(
            out=mask01[:, c * 2:(c + 1) * 2, :],
            in_=dmask[c * 256:(c + 1) * 256, :]
                .rearrange("(a p) i -> p a i", p=128))
    nc.gpsimd.dma_start(out=wo2, in_=dwo.rearrange("h p e m -> p h e m"))

    gamma_sb = beta_sb = None
    if dgamma is not None:
        gamma_sb = persist.tile([128, DM], F32)
        beta_sb = persist.tile([128, DM], F32)
        nc.sync.dma_start(out=gamma_sb, in_=bass.AP(
            tensor=dgamma.tensor, offset=dgamma.offset,
            ap=[[0, 128]] + list(dgamma.ap)))
        nc.sync.dma_start(out=beta_sb, in_=bass.AP(
            tensor=dbeta.tensor, offset=dbeta.offset,
            ap=[[0, 128]] + list(dbeta.ap)))

    # ---------------- Phase A: LN + transpose + QKV projections -------------
    with tc.tile_pool(name="phA", bufs=1) as pA, \
         tc.tile_pool(name="qnTs", bufs=2) as qnTpool, \
         tc.tile_pool(name="qtiles", bufs=2) as qpool, \
         tc.tile_pool(name="stats", bufs=4) as spool, \
         tc.tile_pool(name="psT", bufs=2, space="PSUM") as psT, \
         tc.tile_pool(name="psQK", bufs=3, space="PSUM") as psQK:

        wv_sb = pA.tile([128, 4, 2, HD], FP8)

        hv4 = hv2.rearrange("p a e h x -> p (a e h) x", x=HVP)
        nc.vector.memset(hv4[:, :, DH:DH + 1], 1.0)

        for tb in range(4):
            qt = qpool.tile([128, 4, DM], F32, tag="qt")
            qeng = (nc.sync, nc.scalar, nc.sync, nc.scalar)
            for k in range(4):
                rk = slice(tb * 512 + k * 128, tb * 512 + (k + 1) * 128)
                qeng[k].dma_start(out=qt[:, k, :], in_=dq[rk, :])
            if tb == 0:
                for w_sb, dw in ((wq_sb, dwq), (wk_sb, dwk)):
                    nc.scalar.dma_start(
                        out=w_sb, in_=dw.rearrange("c p e m -> p c e m"))
                nc.gpsimd.dma_start(
                    out=wv_sb, in_=dwv.rearrange("c p e m -> p c e m"))
            # LN stats for the 4 row-tiles, batched
            mv4 = spool.tile([128, 4, 2], F32, tag="mv4")
            for k in range(4):
                st = spool.tile([128, 2, 6], F32, tag="st")
                nc.vector.bn_stats(out=st[:, 0, :], in_=qt[:, k, 0:512])
                nc.vector.bn_stats(out=st[:, 1, :], in_=qt[:, k, 512:1024])
                nc.vector.bn_aggr(out=mv4[:, k, :], in_=st)
            std4 = spool.tile([128, 4], F32, tag="std4")
            nc.scalar.activation(out=std4, in_=mv4[:, :, 1], func=Act.Sqrt,
                                 bias=eps_sb, scale=1.0)
            rstd4 = spool.tile([128, 4], F32, tag="rstd4")
            nc.vector.reciprocal(out=rstd4, in_=std4)
            negmr4 = spool.tile([128, 4], F32, tag="negmr4")
            nc.vector.tensor_tensor(out=negmr4, in0=mv4[:, :, 0], in1=rstd4,
                                    op=Alu.mult)
            nc.vector.tensor_scalar_mul(negmr4, negmr4, -1.0)
            qnf = qpool.tile([128, 4, DM], BF16, tag="qnf")
            for k in range(4):
                nc.vector.tensor_scalar(out=qnf[:, k, :], in0=qt[:, k, :],
                                        scalar1=rstd4[:, k:k + 1],
                                        scalar2=negmr4[:, k:k + 1],
                                        op0=Alu.mult, op1=Alu.add)
                if gamma_sb is not None:
                    nc.vector.tensor_tensor(out=qnf[:, k, :], in0=qnf[:, k, :],
                                            in1=gamma_sb, op=Alu.mult)
                    nc.vector.tensor_tensor(out=qnf[:, k, :], in0=qnf[:, k, :],
                                            in1=beta_sb, op=Alu.add)
            rows4 = slice(tb * 512, (tb + 1) * 512)
            nc.gpsimd.dma_start(
                out=dqn[rows4, :].rearrange("(a p) m -> p a m", p=128), in_=qnf)
            # PE transposes (bf16) -> qnT fp8 (ACT copies)
            qnT = qnTall[:, tb]
            for dc in range(NDC):
                pst = psT.tile([128, 512], BF16, tag="pst")
                for k in range(4):
                    nc.tensor.transpose(pst[:, k * 128:(k + 1) * 128],
                                        qnf[:, k, dc * 128:(dc + 1) * 128],
                                        identb)
                nc.scalar.copy(out=qnT[:, dc // 2, dc % 2, :], in_=pst)

            # QKV for this seq block: only head-pair 0 (hc0) now; hc1-3 are
            # emitted interleaved into the first phase-B units
            sc = slice(tb * 512, (tb + 1) * 512)
            for w_sb, dstT, eng in ((wq_sb, hqT, "act"), (wk_sb, hkT, "act")):
                ps = psQK.tile([128, 512], F32, tag="psqk")
                for c in range(4):
                    nc.tensor.matmul(
                        ps,
                        lhsT=w_sb[:, c, :, 0:128],
                        rhs=qnT[:, c, :, :],
                        start=(c == 0), stop=(c == 3), perf_mode=DR)
                if eng == "act":
                    nc.scalar.mul(out=dstT[:, 0, sc], in_=ps,
                                  mul=ALPHA / 16.0)
                else:
                    nc.vector.tensor_scalar_mul(dstT[:, 0, sc], ps,
                                                ALPHA / 16.0)
            for jc in range(4 * tb, 4 * tb + 4):
                jl = jc - 4 * tb
                ps = psQK.tile([128, HD], F32, tag="psv")
                for c in range(4):
                    nc.tensor.matmul(
                        ps,
                        lhsT=qnT[:, c, :, jl * 128:(jl + 1) * 128],
                        rhs=wv_sb[:, c, :, :],
                        start=(c == 0), stop=(c == 3), perf_mode=DR)
                nc.scalar.mul(
                    out=hv2[:, jc // 2, jc % 2, :, 0:DH],
                    in_=ps.rearrange("p (h x) -> p h x", x=DH), mul=1.0 / 16.0)

    # ------------- Phase B: attention + O-projection ------------------------
    # Loop (iq, hp, jc): iq = i-quarter (512 cols), hp = head pair.  The two
    # heads of a pair occupy partition rows 0:64 / 64:128 of hqT/hkT, so their
    # score matmuls run on alternating PE row-tiles and overlap (~124ns each
    # measured vs ~430ns serial).  One [128,1024] psum pair-tile holds both
    # heads' scores for (jc, iq); one exp op covers both (the mask is
    # h-independent).  AV (fp8 DoubleRow over jc pairs) is deferred a few
    # units so the PE never stalls on a just-issued exp.
    with tc.tile_pool(name="phB", bufs=1) as pB, \
         tc.tile_pool(name="sps", bufs=3, space="PSUM") as spsum, \
         tc.tile_pool(name="vps", bufs=1, space="PSUM") as vpsum, \
         tc.tile_pool(name="pp", bufs=4) as ppool, \
         tc.tile_pool(name="stg", bufs=2) as stpool, \
         tc.tile_pool(name="den", bufs=2) as denpool, \
         tc.tile_pool(name="outs", bufs=4) as outpool:
        vecT = pB.tile([128, 4, SEQ], FP8)     # [d-in-chunk, chunk=hp, i]

        s_of = {}
        pa_of = {}
        vab_of = {}

        def unit(t):
            iq, r = divmod(t, 4 * NJC)
            hp, jc = divmod(r, NJC)
            return iq, hp, jc

        def emit_score(t):
            iq, hp, jc = unit(t)
            isl = slice(iq * 512, (iq + 1) * 512)
            s = spsum.tile([128, 1024], F32, tag="s")
            is_dve = jc in DVE_JC
            for hh in range(2):
                prows = slice(hh * 64, hh * 64 + 64)
                nc.tensor.matmul(
                    s[:, hh * 512:(hh + 1) * 512],
                    lhsT=hkT[prows, hp, jc * 128:(jc + 1) * 128],
                    rhs=hqT[prows, hp, isl],
                    start=True, stop=is_dve)
            if not is_dve:
                # accumulate +128*mask01 into both halves (DoubleRow);
                # the global -128 is folded into BIAS_ACT
                for hh in range(2):
                    nc.tensor.matmul(
                        s[:, hh * 512:(hh + 1) * 512],
                        lhsT=negid,
                        rhs=bass.AP(
                            tensor=mask01.tensor,
                            offset=(mask01.offset + jc * SEQ + iq * 512),
                            ap=[list(mask01.ap[0]), [0, 2], [1, 512]]),
                        start=False, stop=True, perf_mode=DR)
            s_of[t] = s

        def emit_exp(t):
            iq, hp, jc = unit(t)
            s = s_of.pop(t)
            if jc % 2 == 0:
                pa = ppool.tile([128, 2, 2, 512], U8, tag="pa")  # [hh, e, i]
                pa_of[(iq, hp, jc // 2)] = pa
            else:
                pa = pa_of[(iq, hp, jc // 2)]
            e = jc % 2
            out_ap = bass.AP(tensor=pa.tensor,
                             offset=pa.offset + e * 512,
                             ap=[list(pa.ap[0]), [2 * 512, 2], [1, 512]])
            s2 = s.rearrange("p (hh i) -> p hh i", hh=2)
            mk = bass.AP(tensor=mask01.tensor,
                         offset=(mask01.offset + jc * SEQ + iq * 512),
                         ap=[list(mask01.ap[0]), [0, 2], [1, 512]])
            if jc in DVE_JC:
                nc.vector.scalar_tensor_tensor(
                    out=out_ap, in0=s2, scalar=B8, in1=mk,
                    op0=Alu.add, op1=Alu.mult)
            else:
                nc.scalar.activation(out=out_ap.bitcast(FP8), in_=s2,
                                     func=Act.Exp, bias=bias_sb,
                                     scale=SCALE_ACT)

        def emit_av(t):
            iq, hp, jc = unit(t)
            jcp = jc // 2
            pa = pa_of.pop((iq, hp, jcp))
            if jcp == 0:
                vab_of[(iq, hp, 0)] = vpsum.tile(
                    [65, 512], F32, tag="vabe", name=f"vabe{iq}_{hp}")
                vab_of[(iq, hp, 1)] = vpsum.tile(
                    [65, 512], F32, tag="vabo", name=f"vabo{iq}_{hp}")
            for hh in range(2):
                h = 2 * hp + hh
                lh = bass.AP(
                    tensor=hv2.tensor,
                    offset=(hv2.offset + jcp * (2 * HPC * HVP) + h * HVP),
                    ap=[list(hv2.ap[0]), [HPC * HVP, 2], [1, 65]])
                nc.tensor.matmul(
                    vab_of[(iq, hp, hh)], lhsT=lh,
                    rhs=pa[:, hh, :, :].bitcast(FP8),
                    start=(jcp == 0), stop=(jcp == NJC // 2 - 1),
                    perf_mode=DR)

        def emit_readout(iq, hp):
            # stage the psum accumulators to SBUF immediately (frees the
            # single-buffered vab psum for the next group's AV), then do the
            # whole normalization chain from SBUF.
            vabe = vab_of.pop((iq, hp, 0))
            vabo = vab_of.pop((iq, hp, 1))
            stga = stpool.tile([65, 2, 512], F32, tag="stga")
            nc.scalar.copy(out=stga[:, 0, :], in_=vabe)
            nc.scalar.copy(out=stga[:, 1, :], in_=vabo)
            den2 = denpool.tile([2, 512], F32, tag="den2")
            nc.sync.dma_start(out=den2[0:1, :], in_=stga[64:65, 0, :])
            nc.sync.dma_start(out=den2[1:2, :], in_=stga[64:65, 1, :])
            rec2 = denpool.tile([2, 512], F32, tag="rec2")
            nc.vector.reciprocal_approx_fast(out=rec2, in_=den2)
            rec2b = denpool.tile([2, 512], BF16, tag="rec2b")
            nc.vector.tensor_scalar_mul(rec2b, rec2, 16.0)
            g = iq * 4 + hp
            nc.sync.dma_start(out=drecip[g], in_=rec2b)
            bc = denpool.tile([64, 2, 512], BF16, tag="bc")
            row = drecip[g]
            nc.sync.dma_start(
                out=bc,
                in_=bass.AP(tensor=row.tensor, offset=row.offset,
                            ap=[[0, 64]] + list(row.ap)))
            isl = slice(iq * 512, (iq + 1) * 512)
            nc.vector.tensor_tensor(out=vecT[0:64, hp, isl],
                                    in0=stga[0:64, 0, :], in1=bc[:, 0, :],
                                    op=Alu.mult)
            stgb = stpool.tile([64, 512], FP8, tag="stgb")
            nc.vector.tensor_tensor(out=stgb, in0=stga[0:64, 1, :],
                                    in1=bc[:, 1, :], op=Alu.mult)
            nc.gpsimd.dma_start(out=vecT[64:128, hp, isl], in_=stgb)

        o_outt = {}

        def emit_o(it, mc):
            if mc == 0:
                o_outt[it] = outpool.tile([128, 2, 512], BF16, tag="outt",
                                          name=f"outt{it}")
            outt = o_outt[it]
            po_t = spsum.tile([128, 1024], F32, tag="s", name=f"po{it}_{mc}")
            po = po_t[:, 0:512]
            isl = slice(it * 128, (it + 1) * 128)
            for half in range(2):
                nc.tensor.matmul(
                    po,
                    lhsT=bass.AP(
                        tensor=vecT.tensor,
                        offset=(vecT.offset + (2 * half) * SEQ + it * 128),
                        ap=[list(vecT.ap[0]), [SEQ, 2], [1, 128]]),
                    rhs=wo2[:, half, :, mc * 512:(mc + 1) * 512],
                    start=(half == 0), stop=(half == 1), perf_mode=DR)
            nc.scalar.mul(out=outt[:, mc, :], in_=po, mul=1.0 / 512.0)
            if mc == 1:
                del o_outt[it]
                nc.gpsimd.dma_start(
                    out=dpart[isl, :],
                    in_=outt.rearrange("p a m -> p (a m)"))

        def emit_proj(w_sb, dstT, eng, hc, tb):
            qnT = qnTall[:, tb]
            sc = slice(tb * 512, (tb + 1) * 512)
            ps_t = spsum.tile([128, 1024], F32, tag="s", name=f"pj{hc}_{tb}")
            ps = ps_t[:, 0:512]
            for c in range(4):
                nc.tensor.matmul(
                    ps, lhsT=w_sb[:, c, :, hc * 128:(hc + 1) * 128],
                    rhs=qnT[:, c, :, :],
                    start=(c == 0), stop=(c == 3), perf_mode=DR)
            if eng == "act":
                nc.scalar.mul(out=dstT[:, hc, sc], in_=ps, mul=ALPHA / 16.0)
            else:
                nc.vector.tensor_scalar_mul(dstT[:, hc, sc], ps, ALPHA / 16.0)

        pend_proj = []
        for hc in range(1, 4):
            for w_sb, dstT, eng in ((wk_sb, hkT, "act"), (wq_sb, hqT, "act")):
                for tb in range(4):
                    pend_proj.append(
                        lambda w=w_sb, d=dstT, e=eng, hc=hc, tb=tb:
                        emit_proj(w, d, e, hc, tb))

        TOT = 4 * 4 * NJC
        PIPE = 3
        DEFER = 2
        av_q = []
        pend_read = []
        pend_o = []
        for t in range(PIPE):
            emit_score(t)
        for t in range(TOT):
            iq, hp, jc = unit(t)
            emit_exp(t)
            if t + PIPE < TOT:
                emit_score(t + PIPE)
            if jc % 2 == 1:
                av_q.append(t)
            while av_q and t - av_q[0] >= DEFER:
                emit_av(av_q.pop(0))
            if jc == NJC - 1:
                pend_read.append(lambda iq=iq, hp=hp: emit_readout(iq, hp))
                if hp == 3:
                    for it in range(iq * 4, iq * 4 + 4):
                        for mc in range(2):
                            pend_o.append(lambda it=it, mc=mc: emit_o(it, mc))
            elif jc == 1 and pend_read:
                # a fresh group is underway: finish the previous group's
                # deferred AVs, then emit its readout
                while av_q and unit(av_q[0])[0:2] != (iq, hp):
                    emit_av(av_q.pop(0))
                pend_read.pop(0)()
            elif pend_proj:
                pend_proj.pop(0)()
                if pend_proj:
                    pend_proj.pop(0)()
            elif pend_o and t % 4 == 3:
                pend_o.pop(0)()
        while av_q:
            emit_av(av_q.pop(0))
        while pend_read:
            pend_read.pop(0)()
        while pend_o:
            pend_o.pop(0)()


_NC_CACHE = {}


def _build(gamma_trivial, repeat=1):
    key = (bool(gamma_trivial), repeat)
    if key in _NC_CACHE:
        return _NC_CACHE[key]
    nc = bacc.Bacc("TRN2", target_bir_lowering=False, debug=False,
                   num_devices=NCORES)
    dq = nc.dram_tensor("q", [SEQ, DM], F32, kind="ExternalInput").ap()
    dmask = nc.dram_tensor("maskt", [SEQ, SEQ], FP8, kind="ExternalInput").ap()
    dwq = nc.dram_tensor("wq", [4, 128, 2, HD], FP8, kind="ExternalInput").ap()
    dwk = nc.dram_tensor("wk", [4, 128, 2, HD], FP8, kind="ExternalInput").ap()
    dwv = nc.dram_tensor("wv", [4, 128, 2, HD], FP8, kind="ExternalInput").ap()
    dwo = nc.dram_tensor("wo", [2, 128, 2, DM], FP8, kind="ExternalInput").ap()
    dident = nc.dram_tensor("ident", [128, 128], BF16, kind="ExternalInput").ap()
    dnegid = nc.dram_tensor("negid", [128, 2, 128], FP8,
                            kind="ExternalInput").ap()
    dgamma = dbeta = None
    if not gamma_trivial:
        dgamma = nc.dram_tensor("gamma", [DM], F32, kind="ExternalInput").ap()
        dbeta = nc.dram_tensor("beta", [DM], F32, kind="ExternalInput").ap()
    dqn = nc.dram_tensor("qn_out", [SEQ, DM], BF16, kind="ExternalOutput").ap()
    dpart = nc.dram_tensor("part_out", [SEQ, DM], BF16,
                           kind="ExternalOutput").ap()
    drecip = nc.dram_tensor("recip_scratch", [16, 2, 512], BF16).ap()
    with tile.TileContext(nc) as tc:
        for _rep in range(repeat):
            with ExitStack() as ctx:
                _mha_tile(ctx, tc, dq, dmask, dwq, dwk, dwv, dwo, dident,
                          dnegid, dgamma, dbeta, dqn, dpart, drecip)
    nc.compile()
    _NC_CACHE[key] = nc
    return nc


def _run(nc, in_maps, **kwargs):
    return run_bass_kernel_spmd(nc, in_maps, list(range(NCORES)), **kwargs)


def make_in_maps(q, attn_mask, Wq, Wk, Wv, Wo, gamma, beta, gamma_trivial):
    bf = ml_dtypes.bfloat16
    f8 = ml_dtypes.float8_e4m3
    q = np.ascontiguousarray(np.asarray(q, dtype=np.float32))
    maskt = np.ascontiguousarray(
        (~np.asarray(attn_mask, dtype=bool)).T.astype(f8))
    Wq = np.asarray(Wq, dtype=np.float32)
    Wk = np.asarray(Wk, dtype=np.float32)
    Wv = np.asarray(Wv, dtype=np.float32)
    Wo = np.asarray(Wo, dtype=np.float32)
    ident = np.eye(128, dtype=bf)
    negid = np.zeros((128, 2, 128), dtype=np.float32)
    negid[:, 0, :] = 128.0 * np.eye(128, dtype=np.float32)
    negid = negid.astype(f8)
    in_maps = []
    for c in range(NCORES):
        b, g = c // 2, c % 2
        cols = slice(g * HD, (g + 1) * HD)

        def w8(w):
            w = (w[:, cols] * 16.0).reshape(4, 2, 128, HD).transpose(0, 2, 1, 3)
            return np.ascontiguousarray(w.astype(f8))

        wo = (Wo[cols, :] * 32.0).reshape(2, 2, 128, DM).transpose(0, 2, 1, 3)
        m = {
            "q": q[b],
            "maskt": maskt,
            "wq": w8(Wq),
            "wk": w8(Wk),
            "wv": w8(Wv),
            "wo": np.ascontiguousarray(wo.astype(f8)),
            "ident": ident,
            "negid": negid,
        }
        if not gamma_trivial:
            m["gamma"] = np.asarray(gamma, dtype=np.float32)
            m["beta"] = np.asarray(beta, dtype=np.float32)
        in_maps.append(m)
    return in_maps


def kernel(q, attn_mask, Wq, Wk, Wv, Wo, gamma, beta):
    gamma_np = np.asarray(gamma, dtype=np.float32)
    beta_np = np.asarray(beta, dtype=np.float32)
    gamma_trivial = bool(np.all(gamma_np == 1.0) and np.all(beta_np == 0.0))
    nc = _build(gamma_trivial)
    in_maps = make_in_maps(q, attn_mask, Wq, Wk, Wv, Wo, gamma_np, beta_np,
                           gamma_trivial)
    res = _run(nc, in_maps).results
    out = np.empty((BSZ, SEQ, DM), dtype=np.float32)
    for b in range(BSZ):
        out[b] = res[2 * b]["qn_out"].astype(np.float32)
        out[b] += res[2 * b]["part_out"].astype(np.float32)
        out[b] += res[2 * b + 1]["part_out"].astype(np.float32)
    return out


if __name__ == "__main__":
    rng = np.random.default_rng(0)
    ins = {
        "q": rng.standard_normal((BSZ, SEQ, DM), dtype=np.float32),
        "attn_mask": rng.integers(0, 2, (SEQ, SEQ)).astype(bool),
        "Wq": rng.standard_normal((DM, NH * DH), dtype=np.float32) * 0.03,
        "Wk": rng.standard_normal((DM, NH * DH), dtype=np.float32) * 0.03,
        "Wv": rng.standard_normal((DM, NH * DH), dtype=np.float32) * 0.03,
        "Wo": rng.standard_normal((NH * DH, DM), dtype=np.float32) * 0.03,
        "gamma": np.ones(DM, np.float32),
        "beta": np.zeros(DM, np.float32),
    }
    out = kernel(**ins)
    print("kernel ran, out shape", out.shape, out.dtype)


# revision 21
# speedup vs baseline: 1.0173x; 1.0173x over previous
"""Trainium2 Bass kernel for pre-LN multi-head self-attention (v4).

Problem shapes (hardcoded): q (4, 2048, 1024) f32, attn_mask (2048, 2048) bool,
Wq/Wk/Wv (1024, 1024) f32, Wo (1024, 1024) f32, gamma/beta (1024,) f32.
N_HEAD=16, D_HEAD=64, pre-layernorm, softmax over the key axis.

Sharding: 8 cores = 4 batches x 2 head-groups (8 heads each). Host sums the
two head-group partials plus qn per batch.

v4 design (vs v2 baseline at 475us):
  - Everything fp8: scores, AV (DoubleRow over jc pairs), O-proj (DoubleRow).
  - hq/hk quantized with alpha = sqrt(8*log2(e)*SCALE) so the score psum is
    exactly 8*log2(e)*s -- the Schraudolph constant for fp8e4m3.  The DVE
    softmax unit is then ONE scalar_tensor_tensor: uint8(psum + 23.549)*mask,
    whose saturating-uint8 result bytes ARE fp8 probabilities exp(s)/16.
  - ACT softmax units run a real Exp to fp8; their mask is pre-applied into
    the score psum by a DoubleRow matmul with lhsT = -128*I and rhs = the
    same fp8 0/1 mask tile (213ns on the PE, no DVE involvement).
  - AV accumulates jc-PAIRS per DoubleRow matmul (pa packed [128,2,512]
    uint8-bitcast-fp8); hv carries a ones column => psum row 64 = denom.
  - Normalization after AV: recip via DRAM-broadcast trick (as v2), vecT is
    fp8 scaled x16; Wo x32; final psum->bf16 copy scales by 1/512.
  - Outputs bf16 (qn + per-core partial); host does the residual sum in f32.
"""

import numpy as np
import ml_dtypes
from contextlib import ExitStack

import concourse.bass as bass
import concourse.tile as tile
from concourse import bacc, mybir
from concourse.bass_utils import run_bass_kernel_spmd

F32 = mybir.dt.float32
BF16 = mybir.dt.bfloat16
FP8 = mybir.dt.float8e4
U8 = mybir.dt.uint8
Alu = mybir.AluOpType
Act = mybir.ActivationFunctionType
DR = mybir.MatmulPerfMode.DoubleRow

BSZ, SEQ, DM = 4, 2048, 1024
NH, DH = 16, 64
HPC = 8              # heads per core
HD = HPC * DH        # 512 = per-core slice of the head dim
NCORES = 8
SCALE = 1.0 / (DH ** 0.5)
LN_EPS = 1e-5

NT = SEQ // 16 // 8  # placeholder (unused)
NJC = SEQ // 128     # 16 j-tiles
NDC = DM // 128      # 8 d_model chunks
K_SCH = 8.0 / np.log(2.0)          # 11.541560: psum = K_SCH * s
ALPHA = float(np.sqrt(K_SCH * SCALE))   # 1.2011224 hq/hk quant scale
B8 = 23.549                         # Schraudolph offset (DVE stt scalar)
BIAS_ACT = float(-np.log(16.0) - 128.0 * np.log(2.0) / 8.0)  # -ln16 - 128/K
SCALE_ACT = float(1.0 / K_SCH)
HVP = 72                            # padded hv row stride (16B-aligned e-step)

# per-16-jc engine pattern: True = DVE fused unit, False = ACT unit
DVE_JC = (0, 1, 2, 4, 5, 6, 8, 9, 10, 12, 14)


def _mha_tile(ctx, tc, dq, dmask, dwq, dwk, dwv, dwo, dident, dnegid,
              dgamma, dbeta, dqn, dpart, drecip):
    nc = tc.nc

    persist = ctx.enter_context(tc.tile_pool(name="persist", bufs=1))
    identb = persist.tile([128, 128], BF16)
    nc.sync.dma_start(out=identb, in_=dident)
    negid = persist.tile([128, 2, 128], FP8)
    nc.sync.dma_start(out=negid, in_=dnegid)
    eps_sb = persist.tile([128, 1], F32)
    nc.vector.memset(eps_sb, LN_EPS)
    bias_sb = persist.tile([128, 1], F32)
    nc.vector.memset(bias_sb, BIAS_ACT)

    hqT = persist.tile([128, 4, SEQ], FP8)     # [d-in-chunk, hc, i]
    hkT = persist.tile([128, 4, SEQ], FP8)
    qnTall = persist.tile([128, 4, 4, 2, 512], FP8)  # [dm-chunk, tb, c, e, i]
    wq_sb = persist.tile([128, 4, 2, HD], FP8)
    wk_sb = persist.tile([128, 4, 2, HD], FP8)
    hv2 = persist.tile([128, NJC // 2, 2, HPC, HVP], FP8)  # [j, jcp, e, h, d]
    mask01 = persist.tile([128, NJC, SEQ], FP8)  # [j-part, jc, i] 0/1
    wo2 = persist.tile([128, 2, 2, DM], FP8)   # [p, half, e, m]

    # big persistent loads spread across the otherwise-idle DMA queues
    for c in range(8):
        nc.gpsimd.dma_start(
            out=mask01[:, c * 2:(c + 1) * 2, :],
            in_=dmask[c * 256:(c + 1) * 256, :]
                .rearrange("(a p) i -> p a i", p=128))
    nc.gpsimd.dma_start(out=wo2, in_=dwo.rearrange("h p e m -> p h e m"))

    gamma_sb = beta_sb = None
    if dgamma is not None:
        gamma_sb = persist.tile([128, DM], F32)
        beta_sb = persist.tile([128, DM], F32)
        nc.sync.dma_start(out=gamma_sb, in_=bass.AP(
            tensor=dgamma.tensor, offset=dgamma.offset,
            ap=[[0, 128]] + list(dgamma.ap)))
        nc.sync.dma_start(out=beta_sb, in_=bass.AP(
            tensor=dbeta.tensor, offset=dbeta.offset,
            ap=[[0, 128]] + list(dbeta.ap)))

    # ---------------- Phase A: LN + transpose + QKV projections -------------
    with tc.tile_pool(name="phA", bufs=1) as pA, \
         tc.tile_pool(name="qnTs", bufs=2) as qnTpool, \
         tc.tile_pool(name="qtiles", bufs=2) as qpool, \
         tc.tile_pool(name="stats", bufs=4) as spool, \
         tc.tile_pool(name="psT", bufs=2, space="PSUM") as psT, \
         tc.tile_pool(name="psQK", bufs=3, space="PSUM") as psQK:

        wv_sb = pA.tile([128, 4, 2, HD], FP8)

        hv4 = hv2.rearrange("p a e h x -> p (a e h) x", x=HVP)
        nc.vector.memset(hv4[:, :, DH:DH + 1], 1.0)

        for tb in range(4):
            qt = qpool.tile([128, 4, DM], F32, tag="qt")
            qeng = (nc.sync, nc.scalar, nc.sync, nc.scalar)
            for k in range(4):
                rk = slice(tb * 512 + k * 128, tb * 512 + (k + 1) * 128)
                qeng[k].dma_start(out=qt[:, k, :], in_=dq[rk, :])
            if tb == 0:
                for w_sb, dw in ((wq_sb, dwq), (wk_sb, dwk), (wv_sb, dwv)):
                    nc.sync.dma_start(
                        out=w_sb, in_=dw.rearrange("c p e m -> p c e m"))
            # LN stats for the 4 row-tiles, batched
            mv4 = spool.tile([128, 4, 2], F32, tag="mv4")
            for k in range(4):
                st = spool.tile([128, 2, 6], F32, tag="st")
                nc.vector.bn_stats(out=st[:, 0, :], in_=qt[:, k, 0:512])
                nc.vector.bn_stats(out=st[:, 1, :], in_=qt[:, k, 512:1024])
                nc.vector.bn_aggr(out=mv4[:, k, :], in_=st)
            std4 = spool.tile([128, 4], F32, tag="std4")
            nc.scalar.activation(out=std4, in_=mv4[:, :, 1], func=Act.Sqrt,
                                 bias=eps_sb, scale=1.0)
            rstd4 = spool.tile([128, 4], F32, tag="rstd4")
            nc.vector.reciprocal(out=rstd4, in_=std4)
            negmr4 = spool.tile([128, 4], F32, tag="negmr4")
            nc.vector.tensor_tensor(out=negmr4, in0=mv4[:, :, 0], in1=rstd4,
                                    op=Alu.mult)
            nc.vector.tensor_scalar_mul(negmr4, negmr4, -1.0)
            qnf = qpool.tile([128, 4, DM], BF16, tag="qnf")
            for k in range(4):
                nc.vector.tensor_scalar(out=qnf[:, k, :], in0=qt[:, k, :],
                                        scalar1=rstd4[:, k:k + 1],
                                        scalar2=negmr4[:, k:k + 1],
                                        op0=Alu.mult, op1=Alu.add)
                if gamma_sb is not None:
                    nc.vector.tensor_tensor(out=qnf[:, k, :], in0=qnf[:, k, :],
                                            in1=gamma_sb, op=Alu.mult)
                    nc.vector.tensor_tensor(out=qnf[:, k, :], in0=qnf[:, k, :],
                                            in1=beta_sb, op=Alu.add)
            rows4 = slice(tb * 512, (tb + 1) * 512)
            nc.gpsimd.dma_start(
                out=dqn[rows4, :].rearrange("(a p) m -> p a m", p=128), in_=qnf)
            # PE transposes (bf16) -> qnT fp8 (ACT copies)
            qnT = qnTall[:, tb]
            for dc in range(NDC):
                pst = psT.tile([128, 512], BF16, tag="pst")
                for k in range(4):
                    nc.tensor.transpose(pst[:, k * 128:(k + 1) * 128],
                                        qnf[:, k, dc * 128:(dc + 1) * 128],
                                        identb)
                nc.scalar.copy(out=qnT[:, dc // 2, dc % 2, :], in_=pst)

            # QKV for this seq block: only head-pair 0 (hc0) now; hc1-3 are
            # emitted interleaved into the first phase-B units
            sc = slice(tb * 512, (tb + 1) * 512)
            for w_sb, dstT, eng in ((wq_sb, hqT, "act"), (wk_sb, hkT, "act")):
                ps = psQK.tile([128, 512], F32, tag="psqk")
                for c in range(4):
                    nc.tensor.matmul(
                        ps,
                        lhsT=w_sb[:, c, :, 0:128],
                        rhs=qnT[:, c, :, :],
                        start=(c == 0), stop=(c == 3), perf_mode=DR)
                if eng == "act":
                    nc.scalar.mul(out=dstT[:, 0, sc], in_=ps,
                                  mul=ALPHA / 16.0)
                else:
                    nc.vector.tensor_scalar_mul(dstT[:, 0, sc], ps,
                                                ALPHA / 16.0)
            for jc in range(4 * tb, 4 * tb + 4):
                jl = jc - 4 * tb
                ps = psQK.tile([128, HD], F32, tag="psv")
                for c in range(4):
                    nc.tensor.matmul(
                        ps,
                        lhsT=qnT[:, c, :, jl * 128:(jl + 1) * 128],
                        rhs=wv_sb[:, c, :, :],
                        start=(c == 0), stop=(c == 3), perf_mode=DR)
                nc.scalar.mul(
                    out=hv2[:, jc // 2, jc % 2, :, 0:DH],
                    in_=ps.rearrange("p (h x) -> p h x", x=DH), mul=1.0 / 16.0)

    # ------------- Phase B: attention + O-projection ------------------------
    # Loop (iq, hp, jc): iq = i-quarter (512 cols), hp = head pair.  The two
    # heads of a pair occupy partition rows 0:64 / 64:128 of hqT/hkT, so their
    # score matmuls run on alternating PE row-tiles and overlap (~124ns each
    # measured vs ~430ns serial).  One [128,1024] psum pair-tile holds both
    # heads' scores for (jc, iq); one exp op covers both (the mask is
    # h-independent).  AV (fp8 DoubleRow over jc pairs) is deferred a few
    # units so the PE never stalls on a just-issued exp.
    with tc.tile_pool(name="phB", bufs=1) as pB, \
         tc.tile_pool(name="sps", bufs=3, space="PSUM") as spsum, \
         tc.tile_pool(name="vps", bufs=1, space="PSUM") as vpsum, \
         tc.tile_pool(name="pp", bufs=4) as ppool, \
         tc.tile_pool(name="stg", bufs=2) as stpool, \
         tc.tile_pool(name="den", bufs=2) as denpool, \
         tc.tile_pool(name="outs", bufs=4) as outpool:
        vecT = pB.tile([128, 4, SEQ], FP8)     # [d-in-chunk, chunk=hp, i]

        s_of = {}
        pa_of = {}
        vab_of = {}

        def unit(t):
            iq, r = divmod(t, 4 * NJC)
            hp, jc = divmod(r, NJC)
            return iq, hp, jc

        def emit_score(t):
            iq, hp, jc = unit(t)
            isl = slice(iq * 512, (iq + 1) * 512)
            s = spsum.tile([128, 1024], F32, tag="s")
            is_dve = jc in DVE_JC
            for hh in range(2):
                prows = slice(hh * 64, hh * 64 + 64)
                nc.tensor.matmul(
                    s[:, hh * 512:(hh + 1) * 512],
                    lhsT=hkT[prows, hp, jc * 128:(jc + 1) * 128],
                    rhs=hqT[prows, hp, isl],
                    start=True, stop=is_dve)
            if not is_dve:
                # accumulate +128*mask01 into both halves (DoubleRow);
                # the global -128 is folded into BIAS_ACT
                for hh in range(2):
                    nc.tensor.matmul(
                        s[:, hh * 512:(hh + 1) * 512],
                        lhsT=negid,
                        rhs=bass.AP(
                            tensor=mask01.tensor,
                            offset=(mask01.offset + jc * SEQ + iq * 512),
                            ap=[list(mask01.ap[0]), [0, 2], [1, 512]]),
                        start=False, stop=True, perf_mode=DR)
            s_of[t] = s

        def emit_exp(t):
            iq, hp, jc = unit(t)
            s = s_of.pop(t)
            if jc % 2 == 0:
                pa = ppool.tile([128, 2, 2, 512], U8, tag="pa")  # [hh, e, i]
                pa_of[(iq, hp, jc // 2)] = pa
            else:
                pa = pa_of[(iq, hp, jc // 2)]
            e = jc % 2
            out_ap = bass.AP(tensor=pa.tensor,
                             offset=pa.offset + e * 512,
                             ap=[list(pa.ap[0]), [2 * 512, 2], [1, 512]])
            s2 = s.rearrange("p (hh i) -> p hh i", hh=2)
            mk = bass.AP(tensor=mask01.tensor,
                         offset=(mask01.offset + jc * SEQ + iq * 512),
                         ap=[list(mask01.ap[0]), [0, 2], [1, 512]])
            if jc in DVE_JC:
                nc.vector.scalar_tensor_tensor(
                    out=out_ap, in0=s2, scalar=B8, in1=mk,
                    op0=Alu.add, op1=Alu.mult)
            else:
                nc.scalar.activation(out=out_ap.bitcast(FP8), in_=s2,
                                     func=Act.Exp, bias=bias_sb,
                                     scale=SCALE_ACT)

        def emit_av(t):
            iq, hp, jc = unit(t)
            jcp = jc // 2
            pa = pa_of.pop((iq, hp, jcp))
            if jcp == 0:
                vab_of[(iq, hp, 0)] = vpsum.tile(
                    [65, 512], F32, tag="vabe", name=f"vabe{iq}_{hp}")
                vab_of[(iq, hp, 1)] = vpsum.tile(
                    [65, 512], F32, tag="vabo", name=f"vabo{iq}_{hp}")
            for hh in range(2):
                h = 2 * hp + hh
                lh = bass.AP(
                    tensor=hv2.tensor,
                    offset=(hv2.offset + jcp * (2 * HPC * HVP) + h * HVP),
                    ap=[list(hv2.ap[0]), [HPC * HVP, 2], [1, 65]])
                nc.tensor.matmul(
                    vab_of[(iq, hp, hh)], lhsT=lh,
                    rhs=pa[:, hh, :, :].bitcast(FP8),
                    start=(jcp == 0), stop=(jcp == NJC // 2 - 1),
                    perf_mode=DR)

        def emit_readout(iq, hp):
            # stage the psum accumulators to SBUF immediately (frees the
            # single-buffered vab psum for the next group's AV), then do the
            # whole normalization chain from SBUF.
            vabe = vab_of.pop((iq, hp, 0))
            vabo = vab_of.pop((iq, hp, 1))
            stga = stpool.tile([65, 2, 512], F32, tag="stga")
            nc.scalar.copy(out=stga[:, 0, :], in_=vabe)
            nc.scalar.copy(out=stga[:, 1, :], in_=vabo)
            den2 = denpool.tile([2, 512], F32, tag="den2")
            nc.sync.dma_start(out=den2[0:1, :], in_=stga[64:65, 0, :])
            nc.sync.dma_start(out=den2[1:2, :], in_=stga[64:65, 1, :])
            rec2 = denpool.tile([2, 512], F32, tag="rec2")
            nc.vector.reciprocal_approx_fast(out=rec2, in_=den2)
            rec2b = denpool.tile([2, 512], BF16, tag="rec2b")
            nc.vector.tensor_scalar_mul(rec2b, rec2, 16.0)
            g = iq * 4 + hp
            nc.sync.dma_start(out=drecip[g], in_=rec2b)
            bc = denpool.tile([64, 2, 512], BF16, tag="bc")
            row = drecip[g]
            nc.sync.dma_start(
                out=bc,
                in_=bass.AP(tensor=row.tensor, offset=row.offset,
                            ap=[[0, 64]] + list(row.ap)))
            isl = slice(iq * 512, (iq + 1) * 512)
            nc.vector.tensor_tensor(out=vecT[0:64, hp, isl],
                                    in0=stga[0:64, 0, :], in1=bc[:, 0, :],
                                    op=Alu.mult)
            stgb = stpool.tile([64, 512], FP8, tag="stgb")
            nc.vector.tensor_tensor(out=stgb, in0=stga[0:64, 1, :],
                                    in1=bc[:, 1, :], op=Alu.mult)
            nc.gpsimd.dma_start(out=vecT[64:128, hp, isl], in_=stgb)

        o_outt = {}

        def emit_o(it, mc):
            if mc == 0:
                o_outt[it] = outpool.tile([128, 2, 512], BF16, tag="outt",
                                          name=f"outt{it}")
            outt = o_outt[it]
            po_t = spsum.tile([128, 1024], F32, tag="s", name=f"po{it}_{mc}")
            po = po_t[:, 0:512]
            isl = slice(it * 128, (it + 1) * 128)
            for half in range(2):
                nc.tensor.matmul(
                    po,
                    lhsT=bass.AP(
                        tensor=vecT.tensor,
                        offset=(vecT.offset + (2 * half) * SEQ + it * 128),
                        ap=[list(vecT.ap[0]), [SEQ, 2], [1, 128]]),
                    rhs=wo2[:, half, :, mc * 512:(mc + 1) * 512],
                    start=(half == 0), stop=(half == 1), perf_mode=DR)
            nc.scalar.mul(out=outt[:, mc, :], in_=po, mul=1.0 / 512.0)
            if mc == 1:
                del o_outt[it]
                nc.gpsimd.dma_start(
                    out=dpart[isl, :],
                    in_=outt.rearrange("p a m -> p (a m)"))

        def emit_proj(w_sb, dstT, eng, hc, tb):
            qnT = qnTall[:, tb]
            sc = slice(tb * 512, (tb + 1) * 512)
            ps_t = spsum.tile([128, 1024], F32, tag="s", name=f"pj{hc}_{tb}")
            ps = ps_t[:, 0:512]
            for c in range(4):
                nc.tensor.matmul(
                    ps, lhsT=w_sb[:, c, :, hc * 128:(hc + 1) * 128],
                    rhs=qnT[:, c, :, :],
                    start=(c == 0), stop=(c == 3), perf_mode=DR)
            if eng == "act":
                nc.scalar.mul(out=dstT[:, hc, sc], in_=ps, mul=ALPHA / 16.0)
            else:
                nc.vector.tensor_scalar_mul(dstT[:, hc, sc], ps, ALPHA / 16.0)

        pend_proj = []
        for hc in range(1, 4):
            for w_sb, dstT, eng in ((wk_sb, hkT, "act"), (wq_sb, hqT, "act")):
                for tb in range(4):
                    pend_proj.append(
                        lambda w=w_sb, d=dstT, e=eng, hc=hc, tb=tb:
                        emit_proj(w, d, e, hc, tb))

        TOT = 4 * 4 * NJC
        PIPE = 3
        DEFER = 2
        av_q = []
        pend_read = []
        pend_o = []
        for t in range(PIPE):
            emit_score(t)
        for t in range(TOT):
            iq, hp, jc = unit(t)
            emit_exp(t)
            if t + PIPE < TOT:
                emit_score(t + PIPE)
            if jc % 2 == 1:
                av_q.append(t)
            while av_q and t - av_q[0] >= DEFER:
                emit_av(av_q.pop(0))
            if jc == NJC - 1:
                pend_read.append(lambda iq=iq, hp=hp: emit_readout(iq, hp))
                if hp == 3:
                    for it in range(iq * 4, iq * 4 + 4):
                        for mc in range(2):
                            pend_o.append(lambda it=it, mc=mc: emit_o(it, mc))
            elif jc == 1 and pend_read:
                # a fresh group is underway: finish the previous group's
                # deferred AVs, then emit its readout
                while av_q and unit(av_q[0])[0:2] != (iq, hp):
                    emit_av(av_q.pop(0))
                pend_read.pop(0)()
            elif pend_proj:
                pend_proj.pop(0)()
                if pend_proj:
                    pend_proj.pop(0)()
            elif pend_o and t % 4 == 3:
                pend_o.pop(0)()
        while av_q:
            emit_av(av_q.pop(0))
        while pend_read:
            pend_read.pop(0)()
        while pend_o:
            pend_o.pop(0)()


_NC_CACHE = {}


def _build(gamma_trivial, repeat=1):
    key = (bool(gamma_trivial), repeat)
    if key in _NC_CACHE:
        return _NC_CACHE[key]
    nc = bacc.Bacc("TRN2", target_bir_lowering=False, debug=False,
                   num_devices=NCORES)
    dq = nc.dram_tensor("q", [SEQ, DM], F32, kind="ExternalInput").ap()
    dmask = nc.dram_tensor("maskt", [SEQ, SEQ], FP8, kind="ExternalInput").ap()
    dwq = nc.dram_tensor("wq", [4, 128, 2, HD], FP8, kind="ExternalInput").ap()
    dwk = nc.dram_tensor("wk", [4, 128, 2, HD], FP8, kind="ExternalInput").ap()
    dwv = nc.dram_tensor("wv", [4, 128, 2, HD], FP8, kind="ExternalInput").ap()
    dwo = nc.dram_tensor("wo", [2, 128, 2, DM], FP8, kind="ExternalInput").ap()
    dident = nc.dram_tensor("ident", [128, 128], BF16, kind="ExternalInput").ap()
    dnegid = nc.dram_tensor("negid", [128, 2, 128], FP8,
                            kind="ExternalInput").ap()
    dgamma = dbeta = None
    if not gamma_trivial:
        dgamma = nc.dram_tensor("gamma", [DM], F32, kind="ExternalInput").ap()
        dbeta = nc.dram_tensor("beta", [DM], F32, kind="ExternalInput").ap()
    dqn = nc.dram_tensor("qn_out", [SEQ, DM], BF16, kind="ExternalOutput").ap()
    dpart = nc.dram_tensor("part_out", [SEQ, DM], BF16,
                           kind="ExternalOutput").ap()
    drecip = nc.dram_tensor("recip_scratch", [16, 2, 512], BF16).ap()
    with tile.TileContext(nc) as tc:
        for _rep in range(repeat):
            with ExitStack() as ctx:
                _mha_tile(ctx, tc, dq, dmask, dwq, dwk, dwv, dwo, dident,
                          dnegid, dgamma, dbeta, dqn, dpart, drecip)
    nc.compile()
    _NC_CACHE[key] = nc
    return nc


def _run(nc, in_maps, **kwargs):
    return run_bass_kernel_spmd(nc, in_maps, list(range(NCORES)), **kwargs)


def make_in_maps(q, attn_mask, Wq, Wk, Wv, Wo, gamma, beta, gamma_trivial):
    bf = ml_dtypes.bfloat16
    f8 = ml_dtypes.float8_e4m3
    q = np.ascontiguousarray(np.asarray(q, dtype=np.float32))
    maskt = np.ascontiguousarray(
        (~np.asarray(attn_mask, dtype=bool)).T.astype(f8))
    Wq = np.asarray(Wq, dtype=np.float32)
    Wk = np.asarray(Wk, dtype=np.float32)
    Wv = np.asarray(Wv, dtype=np.float32)
    Wo = np.asarray(Wo, dtype=np.float32)
    ident = np.eye(128, dtype=bf)
    negid = np.zeros((128, 2, 128), dtype=np.float32)
    negid[:, 0, :] = 128.0 * np.eye(128, dtype=np.float32)
    negid = negid.astype(f8)
    in_maps = []
    for c in range(NCORES):
        b, g = c // 2, c % 2
        cols = slice(g * HD, (g + 1) * HD)

        def w8(w):
            w = (w[:, cols] * 16.0).reshape(4, 2, 128, HD).transpose(0, 2, 1, 3)
            return np.ascontiguousarray(w.astype(f8))

        wo = (Wo[cols, :] * 32.0).reshape(2, 2, 128, DM).transpose(0, 2, 1, 3)
        m = {
            "q": q[b],
            "maskt": maskt,
            "wq": w8(Wq),
            "wk": w8(Wk),
            "wv": w8(Wv),
            "wo": np.ascontiguousarray(wo.astype(f8)),
            "ident": ident,
            "negid": negid,
        }
        if not gamma_trivial:
            m["gamma"] = np.asarray(gamma, dtype=np.float32)
            m["beta"] = np.asarray(beta, dtype=np.float32)
        in_maps.append(m)
    return in_maps


def kernel(q, attn_mask, Wq, Wk, Wv, Wo, gamma, beta):
    gamma_np = np.asarray(gamma, dtype=np.float32)
    beta_np = np.asarray(beta, dtype=np.float32)
    gamma_trivial = bool(np.all(gamma_np == 1.0) and np.all(beta_np == 0.0))
    nc = _build(gamma_trivial)
    in_maps = make_in_maps(q, attn_mask, Wq, Wk, Wv, Wo, gamma_np, beta_np,
                           gamma_trivial)
    res = _run(nc, in_maps).results
    out = np.empty((BSZ, SEQ, DM), dtype=np.float32)
    for b in range(BSZ):
        out[b] = res[2 * b]["qn_out"].astype(np.float32)
        out[b] += res[2 * b]["part_out"].astype(np.float32)
        out[b] += res[2 * b + 1]["part_out"].astype(np.float32)
    return out


if __name__ == "__main__":
    rng = np.random.default_rng(0)
    ins = {
        "q": rng.standard_normal((BSZ, SEQ, DM), dtype=np.float32),
        "attn_mask": rng.integers(0, 2, (SEQ, SEQ)).astype(bool),
        "Wq": rng.standard_normal((DM, NH * DH), dtype=np.float32) * 0.03,
        "Wk": rng.standard_normal((DM, NH * DH), dtype=np.float32) * 0.03,
        "Wv": rng.standard_normal((DM, NH * DH), dtype=np.float32) * 0.03,
        "Wo": rng.standard_normal((NH * DH, DM), dtype=np.float32) * 0.03,
        "gamma": np.ones(DM, np.float32),
        "beta": np.zeros(DM, np.float32),
    }
    out = kernel(**ins)
    print("kernel ran, out shape", out.shape, out.dtype)
